# revision 1
# baseline (speedup 1.0000x reference)
"""Trainium2 Bass kernel for nn_DiscreteAutoencoder (VQ codebook).

Math reformulation (host-precomputed folding):
  reference picks idx = argmin_k ||e - emb_k||^2 with e = relu(x@W1+b1) @ W2 + b2.
  Since e = h@W2 + b2 lives in an affine 64-dim subspace,
    argmin_k d_k = argmax_k ( e . w_k - ||w_k||^2/2 )
                 = argmax_k ( h . V_k + beta_k )
  with V = W2 @ emb.T   [64, 4096]   (precomputed on host in fp64)
       beta_k = b2 . w_k - ||w_k||^2 / 2.
  So the encoder's second matmul and the distance computation collapse into a
  single [B,65] x [65,4096] score matmul; e is never materialized.

Precision: min top-2 score gap across all 16384 rows is 3.3e-4 (scores are
O(250)), so scores need ~fp32 accuracy: fp16 hi/lo split (2 stacked K-tiles):
  A: [h1; h2*2^11] . [V1; V1*2^-11]   (K=128)
  B: [h1; 1; 1]    . [V2; beta1; beta2] (K=66)
error ~1e-5 (h2 pre-scaled by 2^11 to dodge fp16 denormal flush).
Encoder mm (x@W1) must stay exact fp32 (feeds the argmax); decoder matmuls
(zq@dw1, g@w2h) only affect output values, so they run in float32r
(1 cyc/row at N>=512 vs fp32's 4; measured HW rel err ~1.6e-4).

Loop structure: per 512-row chunk j, the encoder feeds scores/argmax for its
4 m-tiles which feed one batched decoder group, so PE (matmuls/transposes),
ACT (PSUM drains), DVE (argmax scans) and DMA pipeline across chunks.

Data-parallel over batch across 8 cores; weights/codebook replicated.
"""

import numpy as np

import concourse.bass as bass
import concourse.mybir as mybir
import concourse.tile as tile
from concourse import bacc
from concourse.bass_utils import run_bass_kernel_spmd
from concourse.masks import make_identity

F32 = mybir.dt.float32
F32R = mybir.dt.float32r
F16 = mybir.dt.float16
U32 = mybir.dt.uint32

P = 128
B, S, L, K, H = 16384, 1024, 256, 4096, 64
NCORES = 8
BC = B // NCORES            # 2048 rows per core
NJ = BC // 512              # 4 batch chunks of 512 (= decoder groups)
NM = BC // P                # 16 m-tiles of 128 rows
NK1 = S // P                # 8 contraction tiles for x @ W1
NQ = 4                      # score quarters per m-tile (1024 wide, 2 banks)

_BUILT = None
LAST_RESULTS = None


def _build_program():
    nc = bacc.Bacc("TRN2", target_bir_lowering=False, debug=False,
                   num_devices=NCORES)

    x_d = nc.dram_tensor("x", [BC, S], F32, kind="ExternalInput").ap()
    w1_d = nc.dram_tensor("w1", [S, H], F32, kind="ExternalInput").ap()
    b1_d = nc.dram_tensor("b1", [H, 1], F32, kind="ExternalInput").ap()
    va_d = nc.dram_tensor("va", [P, K], F16, kind="ExternalInput").ap()
    vb_d = nc.dram_tensor("vb", [H + 2, K], F16, kind="ExternalInput").ap()
    emb_d = nc.dram_tensor("emb", [K, L], F32, kind="ExternalInput").ap()
    dw1_d = nc.dram_tensor("dw1", [L, H], F32R, kind="ExternalInput").ap()
    db1_d = nc.dram_tensor("db1", [H, 1], F32, kind="ExternalInput").ap()
    w2h_d = nc.dram_tensor("w2h", [H + 1, S], F32R, kind="ExternalInput").ap()
    y_d = nc.dram_tensor("y", [BC, S], F32, kind="ExternalOutput").ap()

    RELU = mybir.ActivationFunctionType.Relu
    COPY = mybir.ActivationFunctionType.Copy

    with tile.TileContext(nc) as tc:
        with tc.tile_pool(name="const", bufs=1) as const, \
             tc.tile_pool(name="xin", bufs=6) as xin_p, \
             tc.tile_pool(name="xtr", bufs=2) as xtr_p, \
             tc.tile_pool(name="henc", bufs=2) as henc_p, \
             tc.tile_pool(name="ssb", bufs=3) as ssb_p, \
             tc.tile_pool(name="junk", bufs=1) as junk_p, \
             tc.tile_pool(name="scan", bufs=6) as scan_p, \
             tc.tile_pool(name="zq", bufs=2) as zq_p, \
             tc.tile_pool(name="zqt", bufs=2) as zqt_p, \
             tc.tile_pool(name="gsb", bufs=2) as g_p, \
             tc.tile_pool(name="osb", bufs=3) as osb_p, \
             tc.tile_pool(name="encp", bufs=2, space="PSUM") as encp_p, \
             tc.tile_pool(name="decp", bufs=2, space="PSUM") as decp_p, \
             tc.tile_pool(name="sps", bufs=2, space="PSUM") as sps_p:

            w1_sb = const.tile([P, NK1 * H], F32)
            nc.sync.dma_start(
                w1_sb[:].rearrange("p (k h) -> p k h", k=NK1),
                w1_d.rearrange("(k p) h -> p k h", p=P))
            va_sb = const.tile([P, K], F16)
            vb_sb = const.tile([H + 2, K], F16)
            w2h_sb = const.tile([H + 1, S], F32R)
            dw1_sb = const.tile([P, 2 * H], F32R)
            b1_sb = const.tile([H, 1], F32)
            db1_sb = const.tile([H, 1], F32)
            ident = const.tile([P, P], F32)
            make_identity(nc, ident[:])

            hsA = const.tile([P, BC], F16)
            hsB = const.tile([H + 2, BC], F16)
            nc.vector.memset(hsB[H:H + 2, :], 1.0)

            def encoder_chunk(c0, width, tag_id, interleave=False):
                """h1/h2 split for batch rows [c0, c0+width)."""
                nmt = width // P
                x_ts = []
                for mm in range(nmt):
                    r = c0 + mm * P
                    x_t = xin_p.tile([P, S], F32, tag="xin",
                                     name=f"x_{tag_id}_{mm}")
                    nc.sync.dma_start(x_t[:], x_d[r:r + P, :])
                    x_ts.append(x_t)
                if c0 == 0:
                    # cold consts: needed only from the first score matmul on,
                    # so issue them after the first x tiles
                    nc.sync.dma_start(b1_sb[:], b1_d[:])
                    nc.sync.dma_start(db1_sb[:], db1_d[:])
                    nc.sync.dma_start(va_sb[:], va_d[:])
                    nc.sync.dma_start(vb_sb[:], vb_d[:])
                    nc.sync.dma_start(w2h_sb[:], w2h_d[:])
                    nc.sync.dma_start(
                        dw1_sb[:].rearrange("p (k h) -> p k h", k=2),
                        dw1_d.rearrange("(k p) h -> p k h", p=P))
                xt_j = xtr_p.tile([P, NK1 * width], F32, tag="xt",
                                  name=f"xt_{tag_id}", padded_shape=[P, NK1 * 512])
                for k in range(NK1):
                    tp = encp_p.tile([P, width], F32, tag="encp",
                                     name=f"tp_{tag_id}_{k}",
                                     padded_shape=[P, 512])
                    for mm in range(nmt):
                        nc.tensor.transpose(tp[:, mm * P:(mm + 1) * P],
                                            x_ts[mm][:, k * P:(k + 1) * P],
                                            ident[:])
                    nc.scalar.copy(xt_j[:, k * width:(k + 1) * width], tp[:])
                # nhalf=2 pipelines mm1/relu/split by column halves so the
                # first score matmuls unblock earlier (used for chunk 0)
                nhalf = 4 if interleave else 1
                wh = width // nhalf
                for hf in range(nhalf):
                    hp = encp_p.tile([H, wh], F32, tag="encp",
                                     name=f"hp_{tag_id}_{hf}",
                                     padded_shape=[P, 512])
                    for k in range(NK1):
                        nc.tensor.matmul(
                            hp[:], lhsT=w1_sb[:, k * H:(k + 1) * H],
                            rhs=xt_j[:, k * width + hf * wh:
                                     k * width + (hf + 1) * wh],
                            start=(k == 0), stop=(k == NK1 - 1))
                    jsl = slice(c0 + hf * wh, c0 + (hf + 1) * wh)
                    h32 = henc_p.tile([H, wh], F32, tag="h32",
                                      name=f"h32_{tag_id}_{hf}",
                                      padded_shape=[H, 512])
                    nc.scalar.activation(h32[:], hp[:], RELU, bias=b1_sb[:],
                                         scale=1.0)
                    nc.scalar.copy(hsA[0:H, jsl], h32[:])
                    nc.scalar.copy(hsB[0:H, jsl], h32[:])
                    tmp32 = henc_p.tile([H, wh], F32, tag="tmp32",
                                        name=f"tmp32_{tag_id}_{hf}",
                                        padded_shape=[H, 512])
                    nc.vector.tensor_sub(tmp32[:], h32[:], hsA[0:H, jsl])
                    nc.scalar.activation(hsA[H:P, jsl], tmp32[:], COPY,
                                         bias=0.0, scale=2048.0)

            def emit_scores(m):
                # ------------- scores + argmax for one m-tile -------------
                for _once in range(1):
                    msl = slice(m * P, (m + 1) * P)
                    s_sb = ssb_p.tile([P, K], F32, tag="ssb", name=f"ssb_{m}")
                    m_val = scan_p.tile([P, 1], F32, tag="mval",
                                        name=f"mval_{m}")
                    for q in range(NQ):
                        sp = sps_p.tile([P, 1024], F32, tag="sps",
                                        name=f"sp_{m}_{q}")
                        for n in range(2):
                            nsl = slice((q * 2 + n) * 512,
                                        (q * 2 + n + 1) * 512)
                            nc.tensor.matmul(sp[:, n * 512:(n + 1) * 512],
                                             lhsT=hsA[:, msl],
                                             rhs=va_sb[:, nsl],
                                             start=True, stop=False)
                        for n in range(2):
                            nsl = slice((q * 2 + n) * 512,
                                        (q * 2 + n + 1) * 512)
                            nc.tensor.matmul(sp[:, n * 512:(n + 1) * 512],
                                             lhsT=hsB[:, msl],
                                             rhs=vb_sb[:, nsl],
                                             start=False, stop=True)
                        nc.scalar.copy(s_sb[:, q * 1024:(q + 1) * 1024],
                                       sp[:])
                    junk = junk_p.tile([P, K], F16, tag="junk",
                                       name=f"junk_{m}")
                    nc.vector.tensor_scalar(
                        out=junk[:], in0=s_sb[:], scalar1=1.0,
                        scalar2=None, op0=mybir.AluOpType.mult,
                        op1=mybir.AluOpType.max, accum_out=m_val[:])
                    m8 = scan_p.tile([P, 8], F32, tag="m8", name=f"m8_{m}")
                    nc.vector.tensor_copy(m8[:], m_val[:].to_broadcast([P, 8]))
                    idx8 = scan_p.tile([P, 8], U32, tag="idx8",
                                       name=f"idx8_{m}")
                    nc.vector.max_index(idx8[:], m8[:], s_sb[:])
                return idx8

            # ---- decoder: sub-groups of 2 m-tiles (batched, f32r) ----
            def decode_subgroup(j, sg, idx_tiles):
                W2 = 2 * P  # 256 batch cols per sub-group
                zq_g = zq_p.tile([P, 2 * L], F32, tag="zq",
                                 name=f"zq_{j}_{sg}")
                for mm in range(2):
                    nc.gpsimd.indirect_dma_start(
                        out=zq_g[:, mm * L:(mm + 1) * L], out_offset=None,
                        in_=emb_d[:],
                        in_offset=bass.IndirectOffsetOnAxis(
                            ap=idx_tiles[mm][:, 0:1], axis=0))
                zqt_g = zqt_p.tile([P, 2 * W2], F32R, tag="zqt",
                                   name=f"zqt_{j}_{sg}")
                for lk in range(2):
                    tp2 = decp_p.tile([P, W2], F32, tag="decp",
                                      name=f"tpz_{j}_{sg}_{lk}",
                                      padded_shape=[P, 512])
                    for mm in range(2):
                        nc.tensor.transpose(
                            tp2[:, mm * P:(mm + 1) * P],
                            zq_g[:, mm * L + lk * P: mm * L + (lk + 1) * P],
                            ident[:])
                    nc.scalar.copy(zqt_g[:, lk * W2:(lk + 1) * W2], tp2[:])
                gp = decp_p.tile([H, W2], F32, tag="decp",
                                 name=f"gp_{j}_{sg}", padded_shape=[P, 512])
                for lk in range(2):
                    nc.tensor.matmul(gp[:],
                                     lhsT=dw1_sb[:, lk * H:(lk + 1) * H],
                                     rhs=zqt_g[:, lk * W2:(lk + 1) * W2],
                                     start=(lk == 0), stop=(lk == 1))
                g_sb = g_p.tile([H + 1, W2], F32R, tag="g",
                                name=f"g_{j}_{sg}", padded_shape=[H + 1, 512])
                nc.scalar.activation(g_sb[0:H, :], gp[:], RELU,
                                     bias=db1_sb[:], scale=1.0)
                # ones row via ACT (memset is not a verifier-approved f32r
                # producer): 1.0 = Copy(in*0 + 1)
                nc.scalar.activation(g_sb[H:H + 1, :],
                                     w2h_sb[0:1, 0:W2], COPY,
                                     bias=1.0, scale=0.0)
                for mm in range(2):
                    m = 4 * j + sg * 2 + mm
                    o_sb = osb_p.tile([P, S], F32, tag="osb",
                                      name=f"osb_{m}")
                    for n2 in range(2):
                        op = decp_p.tile([P, 512], F32, tag="decp",
                                         name=f"op_{m}_{n2}")
                        nc.tensor.matmul(
                            op[:], lhsT=g_sb[:, mm * P:(mm + 1) * P],
                            rhs=w2h_sb[:, n2 * 512:(n2 + 1) * 512],
                            start=True, stop=True)
                        if n2 == 0:
                            nc.scalar.copy(o_sb[:, 0:512], op[:])
                        else:
                            nc.vector.tensor_copy(o_sb[:, 512:1024], op[:])
                    nc.sync.dma_start(y_d[m * P:(m + 1) * P, :], o_sb[:])

            # software pipeline: encoder(j+1) and the first score tile of
            # chunk j+1 are emitted before decoder(j), so DVE has scan work
            # across every chunk boundary
            encoder_chunk(0, 512, "0", interleave=True)
            idx = {}
            for m in range(4):
                idx[m] = emit_scores(m)
            for j in range(NJ):
                if j + 1 < NJ:
                    encoder_chunk((j + 1) * 512, 512, str(j + 1))
                decode_subgroup(j, 0, [idx[4 * j], idx[4 * j + 1]])
                decode_subgroup(j, 1, [idx[4 * j + 2], idx[4 * j + 3]])
                if j + 1 < NJ:
                    for mm in range(4):
                        idx[4 * (j + 1) + mm] = emit_scores(4 * (j + 1) + mm)

    nc.compile()
    return nc


def _prep_inputs(inputs):
    """Host-side fp64 precompute + per-core sharding."""
    x = np.asarray(inputs["x"], dtype=np.float32)
    w1 = np.asarray(inputs["enc_w1"], dtype=np.float32)
    b1 = np.asarray(inputs["enc_b1"], dtype=np.float32)
    w2 = np.asarray(inputs["enc_w2"], dtype=np.float64)
    b2 = np.asarray(inputs["enc_b2"], dtype=np.float64)
    emb = np.asarray(inputs["emb"], dtype=np.float32)
    dw1 = np.asarray(inputs["dec_w1"], dtype=np.float32)
    db1 = np.asarray(inputs["dec_b1"], dtype=np.float32)
    dw2 = np.asarray(inputs["dec_w2"], dtype=np.float32)
    db2 = np.asarray(inputs["dec_b2"], dtype=np.float32)

    emb64 = emb.astype(np.float64)
    V = w2 @ emb64.T                                     # [64, K]
    beta = b2 @ emb64.T - 0.5 * np.sum(emb64 * emb64, axis=1)   # [K]

    V1 = V.astype(np.float16)
    V2 = (V - V1.astype(np.float64)).astype(np.float16)
    beta1 = beta.astype(np.float16)
    beta2 = (beta - beta1.astype(np.float64)).astype(np.float16)
    va = np.concatenate([V1, (V1.astype(np.float64) * 2.0 ** -11
                              ).astype(np.float16)], axis=0)    # [128, K]
    vb = np.concatenate([V2, beta1[None, :], beta2[None, :]],
                        axis=0)                                  # [66, K]
    w2h = np.concatenate([dw2, db2[None, :]], axis=0)            # [65, S]

    shared = {
        "w1": np.ascontiguousarray(w1),
        "b1": np.ascontiguousarray(b1.reshape(H, 1)),
        "va": np.ascontiguousarray(va),
        "vb": np.ascontiguousarray(vb),
        "emb": np.ascontiguousarray(emb),
        "dw1": np.ascontiguousarray(dw1),
        "db1": np.ascontiguousarray(db1.reshape(H, 1)),
        "w2h": np.ascontiguousarray(w2h),
    }
    in_maps = []
    for c in range(NCORES):
        m = dict(shared)
        m["x"] = np.ascontiguousarray(x[c * BC:(c + 1) * BC, :])
        in_maps.append(m)
    return in_maps


def kernel(**inputs) -> np.ndarray:
    global _BUILT, LAST_RESULTS
    if _BUILT is None:
        _BUILT = _build_program()
    nc = _BUILT
    in_maps = _prep_inputs(inputs)
    import os
    import time
    trace = bool(int(os.environ.get("KERNEL_TRACE", "0")))
    last_exc = None
    for attempt in range(3):
        try:
            res = run_bass_kernel_spmd(nc, in_maps,
                                       core_ids=list(range(NCORES)),
                                       trace=trace)
            y = np.concatenate([res.results[c]["y"] for c in range(NCORES)],
                               axis=0)
            LAST_RESULTS = res
            return y
        except Exception as e:  # transient NRT_EXEC_UNIT_UNRECOVERABLE seen
            last_exc = e
            try:
                import jax
                jax.clear_caches()
                from jax._src import api as _jax_api
                _jax_api.clear_backends()
            except Exception:
                pass
            time.sleep(2.0)
    raise last_exc



# revision 19
# speedup vs baseline: 1.0235x; 1.0235x over previous
"""Trainium2 Bass kernel for nn_DiscreteAutoencoder (VQ codebook).

Math reformulation (host-precomputed folding):
  reference picks idx = argmin_k ||e - emb_k||^2 with e = relu(x@W1+b1) @ W2 + b2.
  Since e = h@W2 + b2 lives in an affine 64-dim subspace,
    argmin_k d_k = argmax_k ( e . w_k - ||w_k||^2/2 )
                 = argmax_k ( h . V_k + beta_k )
  with V = W2 @ emb.T   [64, 4096]   (precomputed on host in fp64)
       beta_k = b2 . w_k - ||w_k||^2 / 2.
  So the encoder's second matmul and the distance computation collapse into a
  single [B,65] x [65,4096] score matmul; e is never materialized.

Precision: min top-2 score gap across all 16384 rows is 3.3e-4 (scores are
O(250)); a single argmin flip costs ~9e-3 output rel err, so scores need
~fp32 accuracy: fp16 hi/lo split (2 stacked K-tiles):
  A: [h1; h2*2^11] . [V1; V1*2^-11]   (K=128)
  B: [h1; 1; 1]    . [V2; beta1; beta2] (K=66)
error ~1e-5 (h2 pre-scaled by 2^11 to dodge fp16 denormal flush).
Encoder mm (x@W1) stays exact fp32 (feeds the argmax).

Argmax (replaces DVE max_index, which costs ~4.5us per m-tile):
  per 1024-wide PSUM quarter: one drain pass to fp32 SBUF with fused
  accum-max where the engine supports it (DVE/Pool tensor_scalar; ACT
  drains get an auxiliary DVE SBUF max), spread across ACT/DVE/Pool so
  drain bandwidth is parallel.  Then m1 = rowmax over the 4 quarter
  maxes, and ONE fused pass  (s32 >= m1) * iota, accum-add  yields the
  argmax index arithmetically (unique by the min-gap property), split
  between DVE and Pool.  Index -> uint32 -> indirect gather.

Decoder fold: D1h = emb @ dec_w1 is precomputed on host [4096, 64], so the
gather directly fetches relu-input rows; decoder mm1 and the zq transposes
disappear.  g_sb rows [0:64] = relu(D1h[idx].T + db1), row 64 = ones for the
db2 bias; mm2 runs in float32r (1 cyc/row at N>=512).

Data-parallel over batch across 8 cores; weights/codebook replicated.
"""

import numpy as np

import concourse.bass as bass
import concourse.mybir as mybir
import concourse.tile as tile
from concourse import bacc
from concourse.bass_utils import run_bass_kernel_spmd
from concourse.masks import make_identity

F32 = mybir.dt.float32
F32R = mybir.dt.float32r
F16 = mybir.dt.float16
U32 = mybir.dt.uint32
ALU = mybir.AluOpType
AXL = mybir.AxisListType

P = 128
B, S, L, K, H = 16384, 1024, 256, 4096, 64
NCORES = 8
BC = B // NCORES            # 2048 rows per core
NJ = BC // 512              # 4 batch chunks of 512
NM = BC // P                # 16 m-tiles of 128 rows
NK1 = S // P                # 8 contraction tiles for x @ W1
NQ = 4                      # score quarters per m-tile (1024 wide, 2 banks)

# engine assignment per score quarter for the PSUM->SBUF drain.  walrus
# rejects ALU ops (TensorScalarPtr and kin) on Pool, so Pool only copies; the
# row max comes from one wide fused DVE pass over the SBUF copy.
P1_ENGINES = ["scalar", "scalar", "scalar", "scalar"]

_BUILT = None
LAST_RESULTS = None


def _build_program():
    nc = bacc.Bacc("TRN2", target_bir_lowering=False, debug=False,
                   num_devices=NCORES)

    x_d = nc.dram_tensor("x", [BC, S], F32, kind="ExternalInput").ap()
    w1_d = nc.dram_tensor("w1", [S, H], F32, kind="ExternalInput").ap()
    b1_d = nc.dram_tensor("b1", [H, 1], F32, kind="ExternalInput").ap()
    va_d = nc.dram_tensor("va", [P, K], F16, kind="ExternalInput").ap()
    vb_d = nc.dram_tensor("vb", [H + 2, K], F16, kind="ExternalInput").ap()
    d1h_d = nc.dram_tensor("d1h", [K, H], F32, kind="ExternalInput").ap()
    db1_d = nc.dram_tensor("db1", [H, 1], F32, kind="ExternalInput").ap()
    w2h_d = nc.dram_tensor("w2h", [H + 1, S], F32R, kind="ExternalInput").ap()
    y_d = nc.dram_tensor("y", [BC, S], F32, kind="ExternalOutput").ap()

    RELU = mybir.ActivationFunctionType.Relu
    COPY = mybir.ActivationFunctionType.Copy

    with tile.TileContext(nc) as tc:
        with tc.tile_pool(name="const", bufs=1) as const, \
             tc.tile_pool(name="xin", bufs=6) as xin_p, \
             tc.tile_pool(name="xtr", bufs=2) as xtr_p, \
             tc.tile_pool(name="henc", bufs=2) as henc_p, \
             tc.tile_pool(name="s32", bufs=3) as s32_p, \
             tc.tile_pool(name="junk", bufs=1) as junk_p, \
             tc.tile_pool(name="scan", bufs=8) as scan_p, \
             tc.tile_pool(name="zq", bufs=3) as zq_p, \
             tc.tile_pool(name="gsb", bufs=2) as g_p, \
             tc.tile_pool(name="osb", bufs=3) as osb_p, \
             tc.tile_pool(name="encp", bufs=2, space="PSUM") as encp_p, \
             tc.tile_pool(name="decp", bufs=2, space="PSUM") as decp_p, \
             tc.tile_pool(name="sps", bufs=2, space="PSUM") as sps_p:

            w1_sb = const.tile([P, NK1 * H], F32)
            nc.sync.dma_start(
                w1_sb[:].rearrange("p (k h) -> p k h", k=NK1),
                w1_d.rearrange("(k p) h -> p k h", p=P))
            va_sb = const.tile([P, K], F16)
            vb_sb = const.tile([H + 2, K], F16)
            w2h_sb = const.tile([H + 1, S], F32R)
            b1_sb = const.tile([H, 1], F32)
            db1_sb = const.tile([H, 1], F32)
            ident = const.tile([P, P], F32)
            make_identity(nc, ident[:])
            hsA = const.tile([P, BC], F16)
            hsB = const.tile([H + 2, BC], F16)
            nc.vector.memset(hsB[H:H + 2, :], 1.0)
            g_pp = [const.tile([H + 1, P], F32R, name=f"g_pp{i}")
                    for i in range(4)]
            for gt in g_pp:
                # ones row via ACT (memset is not a verifier-approved f32r
                # producer): 1.0 = Copy(in*0 + 1)
                nc.scalar.activation(gt[H:H + 1, :], ident[0:1, 0:P],
                                     COPY, bias=1.0, scale=0.0)

            def encoder_chunk(c0, width, tag_id, interleave=False):
                """h1/h2 split for batch rows [c0, c0+width)."""
                nmt = width // P
                x_ts = []
                for mm in range(nmt):
                    r = c0 + mm * P
                    x_t = xin_p.tile([P, S], F32, tag="xin",
                                     name=f"x_{tag_id}_{mm}")
                    nc.sync.dma_start(x_t[:], x_d[r:r + P, :])
                    x_ts.append(x_t)
                if c0 == 0:
                    nc.sync.dma_start(b1_sb[:], b1_d[:])
                    nc.sync.dma_start(db1_sb[:], db1_d[:])
                    nc.sync.dma_start(va_sb[:], va_d[:])
                    nc.sync.dma_start(vb_sb[:], vb_d[:])
                    nc.sync.dma_start(w2h_sb[:], w2h_d[:])
                xt_j = xtr_p.tile([P, NK1 * width], F32, tag="xt",
                                  name=f"xt_{tag_id}", padded_shape=[P, NK1 * 512])
                for k in range(NK1):
                    tp = encp_p.tile([P, width], F32, tag="encp",
                                     name=f"tp_{tag_id}_{k}",
                                     padded_shape=[P, 512])
                    for mm in range(nmt):
                        nc.tensor.transpose(tp[:, mm * P:(mm + 1) * P],
                                            x_ts[mm][:, k * P:(k + 1) * P],
                                            ident[:])
                    nc.scalar.copy(xt_j[:, k * width:(k + 1) * width], tp[:])
                nhalf = 4 if interleave else 1
                wh = width // nhalf
                for hf in range(nhalf):
                    hp = encp_p.tile([H, wh], F32, tag="encp",
                                     name=f"hp_{tag_id}_{hf}",
                                     padded_shape=[P, 512])
                    for k in range(NK1):
                        nc.tensor.matmul(
                            hp[:], lhsT=w1_sb[:, k * H:(k + 1) * H],
                            rhs=xt_j[:, k * width + hf * wh:
                                     k * width + (hf + 1) * wh],
                            start=(k == 0), stop=(k == NK1 - 1))
                    jsl = slice(c0 + hf * wh, c0 + (hf + 1) * wh)
                    h32 = henc_p.tile([H, wh], F32, tag="h32",
                                      name=f"h32_{tag_id}_{hf}",
                                      padded_shape=[H, 512])
                    nc.scalar.activation(h32[:], hp[:], RELU, bias=b1_sb[:],
                                         scale=1.0)
                    nc.gpsimd.tensor_copy(hsA[0:H, jsl], h32[:])
                    nc.gpsimd.tensor_copy(hsB[0:H, jsl], h32[:])
                    tmp32 = henc_p.tile([H, wh], F32, tag="tmp32",
                                        name=f"tmp32_{tag_id}_{hf}",
                                        padded_shape=[H, 512])
                    nc.vector.tensor_sub(tmp32[:], h32[:], hsA[0:H, jsl])
                    nc.scalar.activation(hsA[H:P, jsl], tmp32[:], COPY,
                                         bias=0.0, scale=2048.0)

            def emit_scores(m):
                """scores + argmax index for one m-tile; returns idx u32."""
                msl = slice(m * P, (m + 1) * P)
                s32 = s32_p.tile([P, K], F32, tag="s32", name=f"s32_{m}")
                for q in range(NQ):
                    sp = sps_p.tile([P, 1024], F32, tag="sps",
                                    name=f"sp_{m}_{q}")
                    for n in range(2):
                        nsl = slice((q * 2 + n) * 512, (q * 2 + n + 1) * 512)
                        nc.tensor.matmul(sp[:, n * 512:(n + 1) * 512],
                                         lhsT=hsA[:, msl], rhs=va_sb[:, nsl],
                                         start=True, stop=False)
                    for n in range(2):
                        nsl = slice((q * 2 + n) * 512, (q * 2 + n + 1) * 512)
                        nc.tensor.matmul(sp[:, n * 512:(n + 1) * 512],
                                         lhsT=hsB[:, msl], rhs=vb_sb[:, nsl],
                                         start=False, stop=True)
                    qsl = slice(q * 1024, (q + 1) * 1024)
                    eng = P1_ENGINES[q]
                    if eng == "scalar":
                        nc.scalar.copy(s32[:, qsl], sp[:])
                    else:
                        nc.gpsimd.tensor_copy(s32[:, qsl], sp[:])
                junk = junk_p.tile([P, K], F16, tag="junk", name="junk")
                m1 = scan_p.tile([P, 1], F32, tag="m1", name=f"m1_{m}")
                nc.vector.tensor_scalar(
                    out=junk[:], in0=s32[:], scalar1=1.0,
                    scalar2=None, op0=ALU.mult, op1=ALU.max,
                    accum_out=m1[:])
                m8 = scan_p.tile([P, 8], F32, tag="m8", name=f"m8_{m}")
                nc.gpsimd.tensor_copy(m8[:], m1[:].to_broadcast([P, 8]))
                idx8 = scan_p.tile([P, 8], U32, tag="idx8", name=f"idx8_{m}")
                nc.vector.max_index(idx8[:], m8[:], s32[:])
                return idx8

            def decode_mtile(m, idx8):
                zg = zq_p.tile([P, H], F32, tag="zq", name=f"zq_{m}")
                nc.gpsimd.indirect_dma_start(
                    out=zg[:], out_offset=None, in_=d1h_d[:],
                    in_offset=bass.IndirectOffsetOnAxis(ap=idx8[:, 0:1],
                                                        axis=0))
                zgt = decp_p.tile([H, P], F32, tag="decp",
                                  name=f"zgt_{m}", padded_shape=[P, 512])
                nc.tensor.transpose(zgt[:], zg[:], ident[:])
                g_sb = g_pp[m % 4]
                nc.scalar.activation(g_sb[0:H, :], zgt[:], RELU,
                                     bias=db1_sb[:], scale=1.0)
                o_sb = osb_p.tile([P, S], F32, tag="osb", name=f"osb_{m}")
                for n2 in range(2):
                    op = decp_p.tile([P, 512], F32, tag="decp",
                                     name=f"op_{m}_{n2}")
                    nc.tensor.matmul(
                        op[:], lhsT=g_sb[:],
                        rhs=w2h_sb[:, n2 * 512:(n2 + 1) * 512],
                        start=True, stop=True)
                    if n2 == 0:
                        nc.scalar.copy(o_sb[:, 0:512], op[:])
                    elif m % 2 == 0:
                        nc.vector.tensor_copy(o_sb[:, 512:1024], op[:])
                    else:
                        nc.scalar.copy(o_sb[:, 512:1024], op[:])
                nc.sync.dma_start(y_d[m * P:(m + 1) * P, :], o_sb[:])

            # software pipeline: encoder(j+1) is emitted before decoder(j), so
            # PE has matmul work while the argmax chain for chunk j drains
            encoder_chunk(0, 512, "0", interleave=True)
            idx = {}
            for m in range(4):
                idx[m] = emit_scores(m)
            for j in range(NJ):
                if j + 1 < NJ:
                    encoder_chunk((j + 1) * 512, 512, str(j + 1))
                for mm in range(4):
                    decode_mtile(4 * j + mm, idx[4 * j + mm])
                    if j + 1 < NJ:
                        idx[4 * (j + 1) + mm] = emit_scores(4 * (j + 1) + mm)

    nc.compile()
    return nc


def _prep_inputs(inputs):
    """Host-side fp64 precompute + per-core sharding."""
    x = np.asarray(inputs["x"], dtype=np.float32)
    w1 = np.asarray(inputs["enc_w1"], dtype=np.float32)
    b1 = np.asarray(inputs["enc_b1"], dtype=np.float32)
    w2 = np.asarray(inputs["enc_w2"], dtype=np.float64)
    b2 = np.asarray(inputs["enc_b2"], dtype=np.float64)
    emb = np.asarray(inputs["emb"], dtype=np.float32)
    dw1 = np.asarray(inputs["dec_w1"], dtype=np.float32)
    db1 = np.asarray(inputs["dec_b1"], dtype=np.float32)
    dw2 = np.asarray(inputs["dec_w2"], dtype=np.float32)
    db2 = np.asarray(inputs["dec_b2"], dtype=np.float32)

    emb64 = emb.astype(np.float64)
    V = w2 @ emb64.T                                     # [64, K]
    beta = b2 @ emb64.T - 0.5 * np.sum(emb64 * emb64, axis=1)   # [K]

    V1 = V.astype(np.float16)
    V2 = (V - V1.astype(np.float64)).astype(np.float16)
    beta1 = beta.astype(np.float16)
    beta2 = (beta - beta1.astype(np.float64)).astype(np.float16)
    va = np.concatenate([V1, (V1.astype(np.float64) * 2.0 ** -11
                              ).astype(np.float16)], axis=0)    # [128, K]
    vb = np.concatenate([V2, beta1[None, :], beta2[None, :]],
                        axis=0)                                  # [66, K]
    w2h = np.concatenate([dw2, db2[None, :]], axis=0)            # [65, S]
    d1h = (emb64 @ dw1.astype(np.float64)).astype(np.float32)    # [K, 64]

    shared = {
        "w1": np.ascontiguousarray(w1),
        "b1": np.ascontiguousarray(b1.reshape(H, 1)),
        "va": np.ascontiguousarray(va),
        "vb": np.ascontiguousarray(vb),
        "d1h": np.ascontiguousarray(d1h),
        "db1": np.ascontiguousarray(db1.reshape(H, 1)),
        "w2h": np.ascontiguousarray(w2h),
    }
    in_maps = []
    for c in range(NCORES):
        m = dict(shared)
        m["x"] = np.ascontiguousarray(x[c * BC:(c + 1) * BC, :])
        in_maps.append(m)
    return in_maps


def kernel(**inputs) -> np.ndarray:
    global _BUILT, LAST_RESULTS
    if _BUILT is None:
        _BUILT = _build_program()
    nc = _BUILT
    in_maps = _prep_inputs(inputs)
    import os
    import time
    trace = bool(int(os.environ.get("KERNEL_TRACE", "0")))
    last_exc = None
    for attempt in range(3):
        try:
            res = run_bass_kernel_spmd(nc, in_maps,
                                       core_ids=list(range(NCORES)),
                                       trace=trace)
            y = np.concatenate([res.results[c]["y"] for c in range(NCORES)],
                               axis=0)
            LAST_RESULTS = res
            return y
        except Exception as e:  # transient NRT_EXEC_UNIT_UNRECOVERABLE seen
            last_exc = e
            try:
                import jax
                jax.clear_caches()
                from jax._src import api as _jax_api
                _jax_api.clear_backends()
            except Exception:
                pass
            time.sleep(2.0)
    raise last_exc


# revision 38
# speedup vs baseline: 1.0413x; 1.0174x over previous
"""Trainium2 Bass kernel for nn_DiscreteAutoencoder (VQ codebook).

Math reformulation (host-precomputed folding):
  reference picks idx = argmin_k ||e - emb_k||^2 with e = relu(x@W1+b1) @ W2 + b2.
  Since e = h@W2 + b2 lives in an affine 64-dim subspace,
    argmin_k d_k = argmax_k ( e . w_k - ||w_k||^2/2 )
                 = argmax_k ( h . V_k + beta_k )
  with V = W2 @ emb.T   [64, 4096]   (precomputed on host in fp64)
       beta_k = b2 . w_k - ||w_k||^2 / 2.
  So the encoder's second matmul and the distance computation collapse into a
  single [B,65] x [65,4096] score matmul; e is never materialized.

Precision: min top-2 score gap across all 16384 rows is 3.3e-4 (scores are
O(250)); a single argmin flip costs ~9e-3 output rel err, so scores need
~fp32 accuracy: fp16 hi/lo split (2 stacked K-tiles):
  A: [h1; h2*2^11] . [V1; V1*2^-11]   (K=128)
  B: [h1; 1; 1]    . [V2; beta1; beta2] (K=66)
error ~1e-5 (h2 pre-scaled by 2^11 to dodge fp16 denormal flush).
Encoder mm (x@W1) stays exact fp32 (feeds the argmax).

Argmax: score quarters drain PSUM->SBUF on ACT (walrus rejects ALU ops and
PSUM reads on Pool, so ACT/DVE are the only drain engines); then one wide
DVE tensor_scalar pass (2x mode, ~2.2us) fuses the fp16 side-copy with an
exact fp32 accum-max, and DVE max_index (~4.3us, no fast mode exists for
it) finds the first exact match.  idx8 col 0 feeds the gather directly.
Engine budget is the whole game here: DVE ~125us / ACT ~104us / PE ~109us
per core; 4 of 8 xt transpose-drains per chunk ride on DVE (k%2==1), output
drains alternate ACT/DVE by m-tile parity, and all SBUF-only copies
(hsA/hsB, m8 broadcast) go to the otherwise idle Pool.

Decoder fold: D1h = emb @ dec_w1 is precomputed on host [4096, 64], so the
gather directly fetches relu-input rows; decoder mm1 and the zq transposes
disappear.  g_sb rows [0:64] = relu(D1h[idx].T + db1), row 64 = ones for the
db2 bias; mm2 runs in float32r (1 cyc/row at N>=512).

Data-parallel over batch across 8 cores; weights/codebook replicated.
"""

import numpy as np

import concourse.bass as bass
import concourse.mybir as mybir
import concourse.tile as tile
from concourse import bacc
from concourse.bass_utils import run_bass_kernel_spmd
from concourse.masks import make_identity

F32 = mybir.dt.float32
F32R = mybir.dt.float32r
F16 = mybir.dt.float16
U32 = mybir.dt.uint32
ALU = mybir.AluOpType
AXL = mybir.AxisListType

P = 128
B, S, L, K, H = 16384, 1024, 256, 4096, 64
NCORES = 8
BC = B // NCORES            # 2048 rows per core
NJ = BC // 512              # 4 batch chunks of 512
NM = BC // P                # 16 m-tiles of 128 rows
NK1 = S // P                # 8 contraction tiles for x @ W1
NQ = 4                      # score quarters per m-tile (1024 wide, 2 banks)

# engine assignment per score quarter for the PSUM->SBUF drain.  walrus
# rejects ALU ops (TensorScalarPtr and kin) on Pool, so Pool only copies; the
# row max comes from one wide fused DVE pass over the SBUF copy.
P1_ENGINES = ["scalar", "scalar", "scalar", "scalar"]

_BUILT = None
LAST_RESULTS = None


def _build_program():
    nc = bacc.Bacc("TRN2", target_bir_lowering=False, debug=False,
                   num_devices=NCORES)

    x_d = nc.dram_tensor("x", [BC, S], F32, kind="ExternalInput").ap()
    w1_d = nc.dram_tensor("w1", [S, H], F32, kind="ExternalInput").ap()
    b1_d = nc.dram_tensor("b1", [H, 1], F32, kind="ExternalInput").ap()
    va_d = nc.dram_tensor("va", [P, K], F16, kind="ExternalInput").ap()
    vb_d = nc.dram_tensor("vb", [H + 2, K], F16, kind="ExternalInput").ap()
    d1h_d = nc.dram_tensor("d1h", [K, H], F32, kind="ExternalInput").ap()
    db1_d = nc.dram_tensor("db1", [H, 1], F32, kind="ExternalInput").ap()
    w2h_d = nc.dram_tensor("w2h", [H + 1, S], F32R, kind="ExternalInput").ap()
    y_d = nc.dram_tensor("y", [BC, S], F32, kind="ExternalOutput").ap()

    RELU = mybir.ActivationFunctionType.Relu
    COPY = mybir.ActivationFunctionType.Copy

    with tile.TileContext(nc) as tc:
        with tc.tile_pool(name="const", bufs=1) as const, \
             tc.tile_pool(name="xin", bufs=6) as xin_p, \
             tc.tile_pool(name="xtr", bufs=2) as xtr_p, \
             tc.tile_pool(name="henc", bufs=2) as henc_p, \
             tc.tile_pool(name="s32", bufs=3) as s32_p, \
             tc.tile_pool(name="junk", bufs=1) as junk_p, \
             tc.tile_pool(name="scan", bufs=8) as scan_p, \
             tc.tile_pool(name="zq", bufs=3) as zq_p, \
             tc.tile_pool(name="gsb", bufs=2) as g_p, \
             tc.tile_pool(name="osb", bufs=3) as osb_p, \
             tc.tile_pool(name="encp", bufs=2, space="PSUM") as encp_p, \
             tc.tile_pool(name="decp", bufs=2, space="PSUM") as decp_p, \
             tc.tile_pool(name="sps", bufs=2, space="PSUM") as sps_p:

            w1_sb = const.tile([P, NK1 * H], F32)
            nc.sync.dma_start(
                w1_sb[:].rearrange("p (k h) -> p k h", k=NK1),
                w1_d.rearrange("(k p) h -> p k h", p=P))
            va_sb = const.tile([P, K], F16)
            vb_sb = const.tile([H + 2, K], F16)
            w2h_sb = const.tile([H + 1, S], F32R)
            b1_sb = const.tile([H, 1], F32)
            db1_sb = const.tile([H, 1], F32)
            ident = const.tile([P, P], F32)
            make_identity(nc, ident[:])
            hsA = const.tile([P, BC], F16)
            hsB = const.tile([H + 2, BC], F16)
            nc.vector.memset(hsB[H:H + 2, :], 1.0)
            g_pp = [const.tile([H + 1, P], F32R, name=f"g_pp{i}")
                    for i in range(4)]
            for gt in g_pp:
                # ones row via ACT (memset is not a verifier-approved f32r
                # producer): 1.0 = Copy(in*0 + 1)
                nc.scalar.activation(gt[H:H + 1, :], ident[0:1, 0:P],
                                     COPY, bias=1.0, scale=0.0)

            def encoder_chunk(c0, width, tag_id, interleave=False):
                """h1/h2 split for batch rows [c0, c0+width)."""
                nmt = width // P
                x_ts = []
                for mm in range(nmt):
                    r = c0 + mm * P
                    x_t = xin_p.tile([P, S], F32, tag="xin",
                                     name=f"x_{tag_id}_{mm}")
                    nc.sync.dma_start(x_t[:], x_d[r:r + P, :])
                    x_ts.append(x_t)
                if c0 == 0:
                    nc.sync.dma_start(b1_sb[:], b1_d[:])
                    nc.sync.dma_start(db1_sb[:], db1_d[:])
                    nc.sync.dma_start(va_sb[:], va_d[:])
                    nc.sync.dma_start(vb_sb[:], vb_d[:])
                    nc.sync.dma_start(w2h_sb[:], w2h_d[:])
                xt_j = xtr_p.tile([P, NK1 * width], F32, tag="xt",
                                  name=f"xt_{tag_id}", padded_shape=[P, NK1 * 512])
                for k in range(NK1):
                    tp = encp_p.tile([P, width], F32, tag="encp",
                                     name=f"tp_{tag_id}_{k}",
                                     padded_shape=[P, 512])
                    for mm in range(nmt):
                        nc.tensor.transpose(tp[:, mm * P:(mm + 1) * P],
                                            x_ts[mm][:, k * P:(k + 1) * P],
                                            ident[:])
                    if k % 2 == 1:
                        nc.vector.tensor_copy(
                            xt_j[:, k * width:(k + 1) * width], tp[:])
                    else:
                        nc.scalar.copy(xt_j[:, k * width:(k + 1) * width],
                                       tp[:])
                nhalf = 4 if interleave else 1
                wh = width // nhalf
                for hf in range(nhalf):
                    hp = encp_p.tile([H, wh], F32, tag="encp",
                                     name=f"hp_{tag_id}_{hf}",
                                     padded_shape=[P, 512])
                    for k in range(NK1):
                        nc.tensor.matmul(
                            hp[:], lhsT=w1_sb[:, k * H:(k + 1) * H],
                            rhs=xt_j[:, k * width + hf * wh:
                                     k * width + (hf + 1) * wh],
                            start=(k == 0), stop=(k == NK1 - 1))
                    jsl = slice(c0 + hf * wh, c0 + (hf + 1) * wh)
                    h32 = henc_p.tile([H, wh], F32, tag="h32",
                                      name=f"h32_{tag_id}_{hf}",
                                      padded_shape=[H, 512])
                    nc.scalar.activation(h32[:], hp[:], RELU, bias=b1_sb[:],
                                         scale=1.0)
                    nc.gpsimd.tensor_copy(hsA[0:H, jsl], h32[:])
                    nc.gpsimd.tensor_copy(hsB[0:H, jsl], h32[:])
                    tmp32 = henc_p.tile([H, wh], F32, tag="tmp32",
                                        name=f"tmp32_{tag_id}_{hf}",
                                        padded_shape=[H, 512])
                    nc.vector.tensor_sub(tmp32[:], h32[:], hsA[0:H, jsl])
                    nc.scalar.activation(hsA[H:P, jsl], tmp32[:], COPY,
                                         bias=0.0, scale=2048.0)

            def emit_scores(m):
                """scores + argmax index for one m-tile; returns idx u32."""
                msl = slice(m * P, (m + 1) * P)
                s32 = s32_p.tile([P, K], F32, tag="s32", name=f"s32_{m}")
                for q in range(NQ):
                    sp = sps_p.tile([P, 1024], F32, tag="sps",
                                    name=f"sp_{m}_{q}")
                    for n in range(2):
                        nsl = slice((q * 2 + n) * 512, (q * 2 + n + 1) * 512)
                        nc.tensor.matmul(sp[:, n * 512:(n + 1) * 512],
                                         lhsT=hsA[:, msl], rhs=va_sb[:, nsl],
                                         start=True, stop=False)
                    for n in range(2):
                        nsl = slice((q * 2 + n) * 512, (q * 2 + n + 1) * 512)
                        nc.tensor.matmul(sp[:, n * 512:(n + 1) * 512],
                                         lhsT=hsB[:, msl], rhs=vb_sb[:, nsl],
                                         start=False, stop=True)
                    qsl = slice(q * 1024, (q + 1) * 1024)
                    eng = P1_ENGINES[q]
                    if eng == "scalar":
                        nc.scalar.copy(s32[:, qsl], sp[:])
                    else:
                        nc.gpsimd.tensor_copy(s32[:, qsl], sp[:])
                junk = junk_p.tile([P, K], F16, tag="junk", name="junk")
                m1 = scan_p.tile([P, 1], F32, tag="m1", name=f"m1_{m}")
                nc.vector.tensor_scalar(
                    out=junk[:], in0=s32[:], scalar1=1.0,
                    scalar2=None, op0=ALU.mult, op1=ALU.max,
                    accum_out=m1[:])
                m8 = scan_p.tile([P, 8], F32, tag="m8", name=f"m8_{m}")
                nc.gpsimd.tensor_copy(m8[:], m1[:].to_broadcast([P, 8]))
                idx8 = scan_p.tile([P, 8], U32, tag="idx8", name=f"idx8_{m}")
                nc.vector.max_index(idx8[:], m8[:], s32[:])
                return idx8

            def decode_mtile(m, idx8):
                zg = zq_p.tile([P, H], F32, tag="zq", name=f"zq_{m}")
                nc.gpsimd.indirect_dma_start(
                    out=zg[:], out_offset=None, in_=d1h_d[:],
                    in_offset=bass.IndirectOffsetOnAxis(ap=idx8[:, 0:1],
                                                        axis=0))
                zgt = decp_p.tile([H, P], F32, tag="decp",
                                  name=f"zgt_{m}", padded_shape=[P, 512])
                nc.tensor.transpose(zgt[:], zg[:], ident[:])
                g_sb = g_pp[m % 4]
                nc.scalar.activation(g_sb[0:H, :], zgt[:], RELU,
                                     bias=db1_sb[:], scale=1.0)
                o_sb = osb_p.tile([P, S], F32, tag="osb", name=f"osb_{m}")
                for n2 in range(2):
                    op = decp_p.tile([P, 512], F32, tag="decp",
                                     name=f"op_{m}_{n2}")
                    nc.tensor.matmul(
                        op[:], lhsT=g_sb[:],
                        rhs=w2h_sb[:, n2 * 512:(n2 + 1) * 512],
                        start=True, stop=True)
                    if n2 == 0:
                        nc.scalar.copy(o_sb[:, 0:512], op[:])
                    elif m % 2 == 0:
                        nc.vector.tensor_copy(o_sb[:, 512:1024], op[:])
                    else:
                        nc.scalar.copy(o_sb[:, 512:1024], op[:])
                nc.sync.dma_start(y_d[m * P:(m + 1) * P, :], o_sb[:])

            # software pipeline: encoder(j+1) is emitted before decoder(j), so
            # PE has matmul work while the argmax chain for chunk j drains
            encoder_chunk(0, 512, "0", interleave=True)
            idx = {}
            for m in range(4):
                idx[m] = emit_scores(m)
            for j in range(NJ):
                if j + 1 < NJ:
                    encoder_chunk((j + 1) * 512, 512, str(j + 1))
                for mm in range(4):
                    decode_mtile(4 * j + mm, idx[4 * j + mm])
                if j + 1 < NJ:
                    for mm in range(4):
                        idx[4 * (j + 1) + mm] = emit_scores(4 * (j + 1) + mm)

    nc.compile()
    return nc


def _prep_inputs(inputs):
    """Host-side fp64 precompute + per-core sharding."""
    x = np.asarray(inputs["x"], dtype=np.float32)
    w1 = np.asarray(inputs["enc_w1"], dtype=np.float32)
    b1 = np.asarray(inputs["enc_b1"], dtype=np.float32)
    w2 = np.asarray(inputs["enc_w2"], dtype=np.float64)
    b2 = np.asarray(inputs["enc_b2"], dtype=np.float64)
    emb = np.asarray(inputs["emb"], dtype=np.float32)
    dw1 = np.asarray(inputs["dec_w1"], dtype=np.float32)
    db1 = np.asarray(inputs["dec_b1"], dtype=np.float32)
    dw2 = np.asarray(inputs["dec_w2"], dtype=np.float32)
    db2 = np.asarray(inputs["dec_b2"], dtype=np.float32)

    emb64 = emb.astype(np.float64)
    V = w2 @ emb64.T                                     # [64, K]
    beta = b2 @ emb64.T - 0.5 * np.sum(emb64 * emb64, axis=1)   # [K]

    V1 = V.astype(np.float16)
    V2 = (V - V1.astype(np.float64)).astype(np.float16)
    beta1 = beta.astype(np.float16)
    beta2 = (beta - beta1.astype(np.float64)).astype(np.float16)
    va = np.concatenate([V1, (V1.astype(np.float64) * 2.0 ** -11
                              ).astype(np.float16)], axis=0)    # [128, K]
    vb = np.concatenate([V2, beta1[None, :], beta2[None, :]],
                        axis=0)                                  # [66, K]
    w2h = np.concatenate([dw2, db2[None, :]], axis=0)            # [65, S]
    d1h = (emb64 @ dw1.astype(np.float64)).astype(np.float32)    # [K, 64]

    shared = {
        "w1": np.ascontiguousarray(w1),
        "b1": np.ascontiguousarray(b1.reshape(H, 1)),
        "va": np.ascontiguousarray(va),
        "vb": np.ascontiguousarray(vb),
        "d1h": np.ascontiguousarray(d1h),
        "db1": np.ascontiguousarray(db1.reshape(H, 1)),
        "w2h": np.ascontiguousarray(w2h),
    }
    in_maps = []
    for c in range(NCORES):
        m = dict(shared)
        m["x"] = np.ascontiguousarray(x[c * BC:(c + 1) * BC, :])
        in_maps.append(m)
    return in_maps


def kernel(**inputs) -> np.ndarray:
    global _BUILT, LAST_RESULTS
    if _BUILT is None:
        _BUILT = _build_program()
    nc = _BUILT
    in_maps = _prep_inputs(inputs)
    import os
    import time
    trace = bool(int(os.environ.get("KERNEL_TRACE", "0")))
    last_exc = None
    for attempt in range(3):
        try:
            res = run_bass_kernel_spmd(nc, in_maps,
                                       core_ids=list(range(NCORES)),
                                       trace=trace)
            y = np.concatenate([res.results[c]["y"] for c in range(NCORES)],
                               axis=0)
            LAST_RESULTS = res
            return y
        except Exception as e:  # transient NRT_EXEC_UNIT_UNRECOVERABLE seen
            last_exc = e
            try:
                import jax
                jax.clear_caches()
                from jax._src import api as _jax_api
                _jax_api.clear_backends()
            except Exception:
                pass
            time.sleep(2.0)
    raise last_exc


# revision 42
# speedup vs baseline: 1.0528x; 1.0110x over previous
"""Trainium2 Bass kernel for nn_DiscreteAutoencoder (VQ codebook).

Math reformulation (host-precomputed folding):
  reference picks idx = argmin_k ||e - emb_k||^2 with e = relu(x@W1+b1) @ W2 + b2.
  Since e = h@W2 + b2 lives in an affine 64-dim subspace,
    argmin_k d_k = argmax_k ( e . w_k - ||w_k||^2/2 )
                 = argmax_k ( h . V_k + beta_k )
  with V = W2 @ emb.T   [64, 4096]   (precomputed on host in fp64)
       beta_k = b2 . w_k - ||w_k||^2 / 2.
  So the encoder's second matmul and the distance computation collapse into a
  single [B,65] x [65,4096] score matmul; e is never materialized.

Precision: min top-2 score gap across all 16384 rows is 3.3e-4 (scores are
O(250)); a single argmin flip costs ~9e-3 output rel err, so scores need
~fp32 accuracy: fp16 hi/lo split (2 stacked K-tiles):
  A: [h1; h2*2^11] . [V1; V1*2^-11]   (K=128)
  B: [h1; 1; 1]    . [V2; beta1; beta2] (K=66)
error ~1e-5 (h2 pre-scaled by 2^11 to dodge fp16 denormal flush).
Encoder mm (x@W1) stays exact fp32 (feeds the argmax).

Argmax: score quarters drain PSUM->SBUF on ACT (walrus rejects ALU ops and
PSUM reads on Pool, so ACT/DVE are the only drain engines); then one wide
DVE tensor_scalar pass (2x mode, ~2.2us) fuses the fp16 side-copy with an
exact fp32 accum-max, and DVE max_index (~4.3us, no fast mode exists for
it) finds the first exact match.  idx8 col 0 feeds the gather directly.
Engine budget is the whole game here: DVE ~125us / ACT ~104us / PE ~109us
per core; 4 of 8 xt transpose-drains per chunk ride on DVE (k%2==1), output
drains alternate ACT/DVE by m-tile parity, and all SBUF-only copies
(hsA/hsB, m8 broadcast) go to the otherwise idle Pool.

Decoder fold: D1h = emb @ dec_w1 is precomputed on host [4096, 64], so the
gather directly fetches relu-input rows; decoder mm1 and the zq transposes
disappear.  g_sb rows [0:64] = relu(D1h[idx].T + db1), row 64 = ones for the
db2 bias; mm2 runs in float32r (1 cyc/row at N>=512).

Data-parallel over batch across 8 cores; weights/codebook replicated.
"""

import numpy as np

import concourse.bass as bass
import concourse.mybir as mybir
import concourse.tile as tile
from concourse import bacc
from concourse.bass_utils import run_bass_kernel_spmd
from concourse.masks import make_identity

F32 = mybir.dt.float32
F32R = mybir.dt.float32r
F16 = mybir.dt.float16
U32 = mybir.dt.uint32
ALU = mybir.AluOpType
AXL = mybir.AxisListType

P = 128
B, S, L, K, H = 16384, 1024, 256, 4096, 64
NCORES = 8
BC = B // NCORES            # 2048 rows per core
NJ = BC // 512              # 4 batch chunks of 512
NM = BC // P                # 16 m-tiles of 128 rows
NK1 = S // P                # 8 contraction tiles for x @ W1
NQ = 4                      # score quarters per m-tile (1024 wide, 2 banks)

# engine assignment per score quarter for the PSUM->SBUF drain.  walrus
# rejects ALU ops (TensorScalarPtr and kin) on Pool, so Pool only copies; the
# row max comes from one wide fused DVE pass over the SBUF copy.
P1_ENGINES = ["scalar", "scalar", "scalar", "scalar"]

_BUILT = None
LAST_RESULTS = None


def _build_program():
    nc = bacc.Bacc("TRN2", target_bir_lowering=False, debug=False,
                   num_devices=NCORES)

    x_d = nc.dram_tensor("x", [BC, S], F32, kind="ExternalInput").ap()
    w1_d = nc.dram_tensor("w1", [S, H], F32, kind="ExternalInput").ap()
    b1_d = nc.dram_tensor("b1", [H, 1], F32, kind="ExternalInput").ap()
    va_d = nc.dram_tensor("va", [P, K], F16, kind="ExternalInput").ap()
    vb_d = nc.dram_tensor("vb", [H + 2, K], F16, kind="ExternalInput").ap()
    d1h_d = nc.dram_tensor("d1h", [K, H], F32, kind="ExternalInput").ap()
    db1_d = nc.dram_tensor("db1", [H, 1], F32, kind="ExternalInput").ap()
    w2h_d = nc.dram_tensor("w2h", [H + 1, S], F32R, kind="ExternalInput").ap()
    y_d = nc.dram_tensor("y", [BC, S], F32, kind="ExternalOutput").ap()

    RELU = mybir.ActivationFunctionType.Relu
    COPY = mybir.ActivationFunctionType.Copy

    with tile.TileContext(nc) as tc:
        with tc.tile_pool(name="const", bufs=1) as const, \
             tc.tile_pool(name="xin", bufs=6) as xin_p, \
             tc.tile_pool(name="xtr", bufs=2) as xtr_p, \
             tc.tile_pool(name="henc", bufs=2) as henc_p, \
             tc.tile_pool(name="s32", bufs=3) as s32_p, \
             tc.tile_pool(name="junk", bufs=1) as junk_p, \
             tc.tile_pool(name="scan", bufs=8) as scan_p, \
             tc.tile_pool(name="zq", bufs=3) as zq_p, \
             tc.tile_pool(name="gsb", bufs=2) as g_p, \
             tc.tile_pool(name="osb", bufs=3) as osb_p, \
             tc.tile_pool(name="encp", bufs=2, space="PSUM") as encp_p, \
             tc.tile_pool(name="decp", bufs=2, space="PSUM") as decp_p, \
             tc.tile_pool(name="sps", bufs=2, space="PSUM") as sps_p:

            w1_sb = const.tile([P, NK1 * H], F32)
            nc.sync.dma_start(
                w1_sb[:].rearrange("p (k h) -> p k h", k=NK1),
                w1_d.rearrange("(k p) h -> p k h", p=P))
            va_sb = const.tile([P, K], F16)
            vb_sb = const.tile([H + 2, K], F16)
            w2h_sb = const.tile([H + 1, S], F32R)
            b1_sb = const.tile([H, 1], F32)
            db1_sb = const.tile([H, 1], F32)
            ident = const.tile([P, P], F32)
            make_identity(nc, ident[:])
            hsA = const.tile([P, BC], F16)
            hsB = const.tile([H + 2, BC], F16)
            nc.vector.memset(hsB[H:H + 2, :], 1.0)
            g_pp = [const.tile([H + 1, P], F32R, name=f"g_pp{i}")
                    for i in range(4)]
            for gt in g_pp:
                # ones row via ACT (memset is not a verifier-approved f32r
                # producer): 1.0 = Copy(in*0 + 1)
                nc.scalar.activation(gt[H:H + 1, :], ident[0:1, 0:P],
                                     COPY, bias=1.0, scale=0.0)

            def encoder_chunk(c0, width, tag_id, interleave=False):
                """h1/h2 split for batch rows [c0, c0+width)."""
                nmt = width // P
                x_ts = []
                for mm in range(nmt):
                    r = c0 + mm * P
                    x_t = xin_p.tile([P, S], F32, tag="xin",
                                     name=f"x_{tag_id}_{mm}")
                    nc.sync.dma_start(x_t[:], x_d[r:r + P, :])
                    x_ts.append(x_t)
                if c0 == 0:
                    nc.sync.dma_start(b1_sb[:], b1_d[:])
                    nc.sync.dma_start(db1_sb[:], db1_d[:])
                    nc.sync.dma_start(va_sb[:], va_d[:])
                    nc.sync.dma_start(vb_sb[:], vb_d[:])
                    nc.sync.dma_start(w2h_sb[:], w2h_d[:])
                xt_j = xtr_p.tile([P, NK1 * width], F32, tag="xt",
                                  name=f"xt_{tag_id}", padded_shape=[P, NK1 * 512])
                for k in range(NK1):
                    tp = encp_p.tile([P, width], F32, tag="encp",
                                     name=f"tp_{tag_id}_{k}",
                                     padded_shape=[P, 512])
                    for mm in range(nmt):
                        nc.tensor.transpose(tp[:, mm * P:(mm + 1) * P],
                                            x_ts[mm][:, k * P:(k + 1) * P],
                                            ident[:])
                    if k % 2 == 1:
                        nc.vector.tensor_copy(
                            xt_j[:, k * width:(k + 1) * width], tp[:])
                    else:
                        nc.scalar.copy(xt_j[:, k * width:(k + 1) * width],
                                       tp[:])
                nhalf = 4 if interleave else 1
                wh = width // nhalf
                for hf in range(nhalf):
                    hp = encp_p.tile([H, wh], F32, tag="encp",
                                     name=f"hp_{tag_id}_{hf}",
                                     padded_shape=[P, 512])
                    for k in range(NK1):
                        nc.tensor.matmul(
                            hp[:], lhsT=w1_sb[:, k * H:(k + 1) * H],
                            rhs=xt_j[:, k * width + hf * wh:
                                     k * width + (hf + 1) * wh],
                            start=(k == 0), stop=(k == NK1 - 1))
                    jsl = slice(c0 + hf * wh, c0 + (hf + 1) * wh)
                    h32 = henc_p.tile([H, wh], F32, tag="h32",
                                      name=f"h32_{tag_id}_{hf}",
                                      padded_shape=[H, 512])
                    nc.scalar.activation(h32[:], hp[:], RELU, bias=b1_sb[:],
                                         scale=1.0)
                    nc.gpsimd.tensor_copy(hsA[0:H, jsl], h32[:])
                    nc.gpsimd.tensor_copy(hsB[0:H, jsl], h32[:])
                    tmp32 = henc_p.tile([H, wh], F32, tag="tmp32",
                                        name=f"tmp32_{tag_id}_{hf}",
                                        padded_shape=[H, 512])
                    nc.vector.tensor_sub(tmp32[:], h32[:], hsA[0:H, jsl])
                    nc.scalar.activation(hsA[H:P, jsl], tmp32[:], COPY,
                                         bias=0.0, scale=2048.0)

            def emit_scores(m):
                """scores + argmax index for one m-tile; returns idx u32."""
                msl = slice(m * P, (m + 1) * P)
                s32 = s32_p.tile([P, K], F32, tag="s32", name=f"s32_{m}")
                for q in range(NQ):
                    sp = sps_p.tile([P, 1024], F32, tag="sps",
                                    name=f"sp_{m}_{q}")
                    for n in range(2):
                        nsl = slice((q * 2 + n) * 512, (q * 2 + n + 1) * 512)
                        nc.tensor.matmul(sp[:, n * 512:(n + 1) * 512],
                                         lhsT=hsA[:, msl], rhs=va_sb[:, nsl],
                                         start=True, stop=False)
                    for n in range(2):
                        nsl = slice((q * 2 + n) * 512, (q * 2 + n + 1) * 512)
                        nc.tensor.matmul(sp[:, n * 512:(n + 1) * 512],
                                         lhsT=hsB[:, msl], rhs=vb_sb[:, nsl],
                                         start=False, stop=True)
                    qsl = slice(q * 1024, (q + 1) * 1024)
                    eng = P1_ENGINES[q]
                    if eng == "scalar":
                        nc.scalar.copy(s32[:, qsl], sp[:])
                    else:
                        nc.gpsimd.tensor_copy(s32[:, qsl], sp[:])
                junk = junk_p.tile([P, K], F16, tag="junk", name="junk")
                m1 = scan_p.tile([P, 1], F32, tag="m1", name=f"m1_{m}")
                nc.vector.tensor_scalar(
                    out=junk[:], in0=s32[:], scalar1=1.0,
                    scalar2=None, op0=ALU.mult, op1=ALU.max,
                    accum_out=m1[:])
                m8 = scan_p.tile([P, 8], F32, tag="m8", name=f"m8_{m}")
                nc.gpsimd.tensor_copy(m8[:], m1[:].to_broadcast([P, 8]))
                idx8 = scan_p.tile([P, 8], U32, tag="idx8", name=f"idx8_{m}")
                nc.vector.max_index(idx8[:], m8[:], s32[:])
                return idx8

            def decode_mtile(m, idx8):
                zg = zq_p.tile([P, H], F32, tag="zq", name=f"zq_{m}")
                nc.gpsimd.indirect_dma_start(
                    out=zg[:], out_offset=None, in_=d1h_d[:],
                    in_offset=bass.IndirectOffsetOnAxis(ap=idx8[:, 0:1],
                                                        axis=0))
                zgt = decp_p.tile([H, P], F32, tag="decp",
                                  name=f"zgt_{m}", padded_shape=[P, 512])
                nc.tensor.transpose(zgt[:], zg[:], ident[:])
                g_sb = g_pp[m % 4]
                nc.scalar.activation(g_sb[0:H, :], zgt[:], RELU,
                                     bias=db1_sb[:], scale=1.0)
                o_sb = osb_p.tile([P, S], F32, tag="osb", name=f"osb_{m}")
                for n2 in range(2):
                    op = decp_p.tile([P, 512], F32, tag="decp",
                                     name=f"op_{m}_{n2}")
                    nc.tensor.matmul(
                        op[:], lhsT=g_sb[:],
                        rhs=w2h_sb[:, n2 * 512:(n2 + 1) * 512],
                        start=True, stop=True)
                    if n2 == 0:
                        nc.scalar.copy(o_sb[:, 0:512], op[:])
                    elif m % 2 == 0 and m < 12:
                        nc.vector.tensor_copy(o_sb[:, 512:1024], op[:])
                    else:
                        nc.scalar.copy(o_sb[:, 512:1024], op[:])
                nc.sync.dma_start(y_d[m * P:(m + 1) * P, :], o_sb[:])

            # software pipeline: encoder(j+1) is emitted before decoder(j), so
            # PE has matmul work while the argmax chain for chunk j drains
            encoder_chunk(0, 512, "0", interleave=True)
            idx = {}
            for m in range(4):
                idx[m] = emit_scores(m)
            for j in range(NJ):
                if j + 1 < NJ:
                    encoder_chunk((j + 1) * 512, 512, str(j + 1))
                if j + 1 == NJ - 1:
                    # last-chunk scores ahead of this chunk's decodes so the
                    # final DVE chain starts as early as possible
                    for mm in range(4):
                        idx[4 * (j + 1) + mm] = emit_scores(4 * (j + 1) + mm)
                    for mm in range(4):
                        decode_mtile(4 * j + mm, idx[4 * j + mm])
                else:
                    for mm in range(4):
                        decode_mtile(4 * j + mm, idx[4 * j + mm])
                    if j + 1 < NJ:
                        for mm in range(4):
                            idx[4 * (j + 1) + mm] = emit_scores(
                                4 * (j + 1) + mm)

    nc.compile()
    return nc


def _prep_inputs(inputs):
    """Host-side fp64 precompute + per-core sharding."""
    x = np.asarray(inputs["x"], dtype=np.float32)
    w1 = np.asarray(inputs["enc_w1"], dtype=np.float32)
    b1 = np.asarray(inputs["enc_b1"], dtype=np.float32)
    w2 = np.asarray(inputs["enc_w2"], dtype=np.float64)
    b2 = np.asarray(inputs["enc_b2"], dtype=np.float64)
    emb = np.asarray(inputs["emb"], dtype=np.float32)
    dw1 = np.asarray(inputs["dec_w1"], dtype=np.float32)
    db1 = np.asarray(inputs["dec_b1"], dtype=np.float32)
    dw2 = np.asarray(inputs["dec_w2"], dtype=np.float32)
    db2 = np.asarray(inputs["dec_b2"], dtype=np.float32)

    emb64 = emb.astype(np.float64)
    V = w2 @ emb64.T                                     # [64, K]
    beta = b2 @ emb64.T - 0.5 * np.sum(emb64 * emb64, axis=1)   # [K]

    V1 = V.astype(np.float16)
    V2 = (V - V1.astype(np.float64)).astype(np.float16)
    beta1 = beta.astype(np.float16)
    beta2 = (beta - beta1.astype(np.float64)).astype(np.float16)
    va = np.concatenate([V1, (V1.astype(np.float64) * 2.0 ** -11
                              ).astype(np.float16)], axis=0)    # [128, K]
    vb = np.concatenate([V2, beta1[None, :], beta2[None, :]],
                        axis=0)                                  # [66, K]
    w2h = np.concatenate([dw2, db2[None, :]], axis=0)            # [65, S]
    d1h = (emb64 @ dw1.astype(np.float64)).astype(np.float32)    # [K, 64]

    shared = {
        "w1": np.ascontiguousarray(w1),
        "b1": np.ascontiguousarray(b1.reshape(H, 1)),
        "va": np.ascontiguousarray(va),
        "vb": np.ascontiguousarray(vb),
        "d1h": np.ascontiguousarray(d1h),
        "db1": np.ascontiguousarray(db1.reshape(H, 1)),
        "w2h": np.ascontiguousarray(w2h),
    }
    in_maps = []
    for c in range(NCORES):
        m = dict(shared)
        m["x"] = np.ascontiguousarray(x[c * BC:(c + 1) * BC, :])
        in_maps.append(m)
    return in_maps


def kernel(**inputs) -> np.ndarray:
    global _BUILT, LAST_RESULTS
    if _BUILT is None:
        _BUILT = _build_program()
    nc = _BUILT
    in_maps = _prep_inputs(inputs)
    import os
    import time
    trace = bool(int(os.environ.get("KERNEL_TRACE", "0")))
    last_exc = None
    for attempt in range(3):
        try:
            res = run_bass_kernel_spmd(nc, in_maps,
                                       core_ids=list(range(NCORES)),
                                       trace=trace)
            y = np.concatenate([res.results[c]["y"] for c in range(NCORES)],
                               axis=0)
            LAST_RESULTS = res
            return y
        except Exception as e:  # transient NRT_EXEC_UNIT_UNRECOVERABLE seen
            last_exc = e
            try:
                import jax
                jax.clear_caches()
                from jax._src import api as _jax_api
                _jax_api.clear_backends()
            except Exception:
                pass
            time.sleep(2.0)
    raise last_exc
